# revision 36
# baseline (speedup 1.0000x reference)
"""Trainium2 Bass kernel for nn_ContinuumMemoryCell (scatter_memory).

Data-parallel over batch B across 8 NeuronCores. Device does the three
B-sized matmuls (error = x @ (V_w.T - M) in bf16, y_pred = x @ M in fp8
DoubleRow, and the Hebbian partial dp_i = error_i.T @ x_i in bf16) plus
the fused elementwise output out = y_pred + mix * error. Everything
O(D*H) or smaller (gate vectors, sigmoid means, the final new_M AXPY,
shard stitching) happens on host.

DMA strategy (measured): one HWDGE ring moves ~425 GB/s with big
transfers and parallel rings don't add bandwidth (HBM-bound), so all
input loads ride the sync ring as a few large chunks in need order;
out writes ride scalar's ring; dp writes ride gpsimd.
"""

import sys

if "/opt/trn_rl_repo" not in sys.path:
    sys.path.insert(0, "/opt/trn_rl_repo")

import numpy as np
import ml_dtypes

B, D, H = 16384, 1024, 1024
NCORES = 8
BL = B // NCORES          # 2048 batch rows per core
P = 128                   # partitions
NB = BL // P              # 16 b-tiles per core
NK = D // P               # 8 k-tiles (contraction over d)
NH = H // P               # 8 h-tiles (dp output rows)
FD = 512                  # matmul moving free-dim (one PSUM bank of f32)
OUT_SCALE = 4096.0        # 2^12: lifts M into fp8e4m3 normal range

# k-tile groups for the pass-A input stream: small leading chunks so the
# err pass starts early, larger trailing chunks to amortize per-transfer
# fixed cost.
KGROUPS = [[0], [1], [2, 3], [4, 5], [6, 7]]

_CACHE = {}


def _build():
    """Build + compile the SPMD Bass program (once per process)."""
    if "nc" in _CACHE:
        return _CACHE["nc"]

    import concourse.bacc as bacc
    import concourse.mybir as mybir
    import concourse.tile as tile

    bf16 = mybir.dt.bfloat16
    f32 = mybir.dt.float32
    fp8 = mybir.dt.float8e4

    nc = bacc.Bacc("TRN2", target_bir_lowering=False, debug=False,
                   num_devices=NCORES)

    # wx: per k-group [xT rows | we rows] pre-interleaved on host so the
    # whole pass-A stream is contiguous column chunks of one tensor.
    WXCOLS = NK * (BL + H)
    wx_d = nc.dram_tensor("wx", [P, WXCOLS], bf16, kind="ExternalInput")
    xn_d = nc.dram_tensor("xn", [BL, D], bf16, kind="ExternalInput")
    # x and M*2^12 in fp8 e4m3, pre-paired along d for DoubleRow matmuls:
    # row k' of the logical [D/2, 2, *] holds d = 2k' and 2k'+1.
    x8_d = nc.dram_tensor("x8", [D // 2, 2 * BL], fp8, kind="ExternalInput")
    m8_d = nc.dram_tensor("m8", [D // 2, 2 * D], fp8, kind="ExternalInput")
    mx_d = nc.dram_tensor("mx", [P, NB], f32, kind="ExternalInput")
    out_d = nc.dram_tensor("out", [BL, H], f32, kind="ExternalOutput")
    dp_d = nc.dram_tensor("dp", [H, D], f32, kind="ExternalOutput")

    with tile.TileContext(nc) as tc:
        with (
            tc.tile_pool(name="big", bufs=1) as big,
            tc.tile_pool(name="work", bufs=4) as work,
            tc.tile_pool(name="ps", bufs=4, space="PSUM") as ps,
        ):
            wx_g = [big.tile([P, len(g) * (BL + H)], bf16, tag=f"wxg{j}",
                             name=f"wxg{j}") for j, g in enumerate(KGROUPS)]
            xn_sb = big.tile([P, NB * D], bf16, tag="xn", name="xn")
            x8_sb = big.tile([P, 4 * 2 * BL], fp8, tag="x8", name="x8")
            m8_sb = big.tile([P, 4 * 2 * D], fp8, tag="m8", name="m8")
            err_sb = [big.tile([P, H], bf16, tag=f"err{i}", name=f"err{i}")
                      for i in range(NB)]
            mx_sb = big.tile([P, NB], f32, tag="mx", name="mx")

            # per-k views into the wx group tiles
            xT_v, we_v = {}, {}
            for j, g in enumerate(KGROUPS):
                n = len(g)
                for jj, k in enumerate(g):
                    xT_v[k] = (wx_g[j], jj * BL)
                    we_v[k] = (wx_g[j], n * BL + jj * H)

            # All input loads on the sync HWDGE ring, need order.
            off = 0
            for j, g in enumerate(KGROUPS):
                ncols = len(g) * (BL + H)
                nc.sync.dma_start(wx_g[j][:], wx_d[:, off:off + ncols])
                off += ncols
            nc.gpsimd.dma_start(mx_sb[:], mx_d[:])
            nc.sync.dma_start(
                x8_sb[:].rearrange("p (g c) -> p g c", g=4),
                x8_d.rearrange("(g p) c -> p g c", p=P))
            nc.sync.dma_start(
                m8_sb[:].rearrange("p (g c) -> p g c", g=4),
                m8_d.rearrange("(g p) c -> p g c", p=P))
            nc.sync.dma_start(
                xn_sb[:].rearrange("p (t d) -> p t d", t=NB),
                xn_d.rearrange("(t p) d -> p t d", p=P))

            # Phase 1a: err = x @ (V_w.T - M), stored bf16 in SBUF.
            for i in range(NB):
                pe = ps.tile([P, H], f32, tag="acc", name=f"pe{i}")
                for k in range(NK):
                    xt, xo = xT_v[k]
                    wt, wo = we_v[k]
                    lhs = xt[:, xo + i * P:xo + (i + 1) * P]
                    st, sp = (k == 0), (k == NK - 1)
                    for h2 in range(2):
                        nc.tensor.matmul(
                            pe[:, h2 * FD:(h2 + 1) * FD], lhs,
                            wt[:, wo + h2 * FD:wo + (h2 + 1) * FD],
                            start=st, stop=sp)
                nc.scalar.activation(err_sb[i][:], pe[:],
                                     mybir.ActivationFunctionType.Copy)

            # Phase 1b: y*2^12 = x @ (M*2^12) in fp8 DoubleRow (2 d-rows
            # per PE cell, half the instructions); epilogue out*2^12 =
            # err*(mix*2^12) + y*2^12, rescaled on host.
            x8_4 = x8_sb[:].rearrange("p (g two b) -> p g two b", g=4, two=2)
            m8_4 = m8_sb[:].rearrange("p (g two n) -> p g two n", g=4, two=2)
            for i in range(NB):
                py = ps.tile([P, D], f32, tag="acc", name=f"py{i}")
                for kg in range(4):
                    lhs3 = x8_4[:, kg, :, i * P:(i + 1) * P]
                    st, sp = (kg == 0), (kg == 3)
                    for h2 in range(2):
                        nc.tensor.matmul(
                            py[:, h2 * FD:(h2 + 1) * FD], lhs3,
                            m8_4[:, kg, :, h2 * FD:(h2 + 1) * FD],
                            start=st, stop=sp,
                            perf_mode=mybir.MatmulPerfMode.DoubleRow)
                o = work.tile([P, D], f32, tag="o", name=f"o{i}", bufs=6)
                for h2 in range(2):
                    nc.vector.scalar_tensor_tensor(
                        o[:, h2 * FD:(h2 + 1) * FD],
                        err_sb[i][:, h2 * FD:(h2 + 1) * FD],
                        mx_sb[:, i:i + 1],
                        py[:, h2 * FD:(h2 + 1) * FD],
                        mybir.AluOpType.mult, mybir.AluOpType.add)
                nc.scalar.dma_start(out_d[i * P:(i + 1) * P, :], o[:])

            # Phase 2: dp[h-tile] = sum_b err[b, h-tile].T @ x[b, :]
            for t in range(NH):
                pd = ps.tile([P, D], f32, tag="acc", name=f"pd{t}")
                for i in range(NB):
                    lhs = err_sb[i][:, t * P:(t + 1) * P]
                    st, sp = (i == 0), (i == NB - 1)
                    for h2 in range(2):
                        nc.tensor.matmul(
                            pd[:, h2 * FD:(h2 + 1) * FD], lhs,
                            xn_sb[:, i * D + h2 * FD:i * D + (h2 + 1) * FD],
                            start=st, stop=sp)
                dpt = work.tile([P, D], f32, tag="dpt", name=f"dpt{t}")
                if t < NH - 1:
                    for h2 in range(2):
                        nc.vector.tensor_copy(dpt[:, h2 * FD:(h2 + 1) * FD],
                                              pd[:, h2 * FD:(h2 + 1) * FD])
                        nc.gpsimd.dma_start(
                            dp_d[t * P:(t + 1) * P, h2 * FD:(h2 + 1) * FD],
                            dpt[:, h2 * FD:(h2 + 1) * FD])
                else:
                    # Last h-tile gates the kernel tail: drain in quarters
                    # across both idle HWDGE rings to shorten the chain.
                    Q = FD // 2
                    dp_eng = [nc.sync, nc.scalar]
                    for q in range(4):
                        nc.vector.tensor_copy(dpt[:, q * Q:(q + 1) * Q],
                                              pd[:, q * Q:(q + 1) * Q])
                        dp_eng[q % 2].dma_start(
                            dp_d[t * P:(t + 1) * P, q * Q:(q + 1) * Q],
                            dpt[:, q * Q:(q + 1) * Q])

    nc.compile()
    _CACHE["nc"] = nc
    return nc


def _prepare(inputs):
    """Host-side preprocessing: shard + dtype-convert + gate math."""
    x = np.asarray(inputs["x"], np.float32)
    V_w = np.asarray(inputs["V_w"], np.float32)
    M = np.asarray(inputs["M"], np.float32)
    fg_w = np.asarray(inputs["fg_w"], np.float32)
    fg_b = np.asarray(inputs["fg_b"], np.float32)
    ug_w = np.asarray(inputs["ug_w"], np.float32)
    ug_b = np.asarray(inputs["ug_b"], np.float32)
    sm_w = np.asarray(inputs["sm_w"], np.float32)
    sm_b = np.asarray(inputs["sm_b"], np.float32)

    bf16 = ml_dtypes.bfloat16
    fp8 = ml_dtypes.float8_e4m3
    VT = V_w.T                                # [D, H]
    we = np.ascontiguousarray(VT - M).astype(bf16)
    m8 = np.clip(M * OUT_SCALE, -240, 240).astype(fp8).reshape(D // 2, 2 * D)

    # we parts per k-group, shared across cores: [P, n*H] each
    we_parts = []
    for g in KGROUPS:
        n = len(g)
        wp = we[g[0] * P:(g[0] + n) * P].reshape(n, P, H)
        we_parts.append(wp.transpose(1, 0, 2).reshape(P, n * H))

    # v @ a == x @ (V_w.T @ a): collapse each gate to one D-vector on x.
    c_f = VT @ fg_w[0, :H] + fg_w[0, H:]
    c_u = VT @ ug_w[0, :H] + ug_w[0, H:]
    c_m = VT @ sm_w[0]
    logits = x @ np.stack([c_f, c_u, c_m], axis=1)       # [B, 3]
    sig = 1.0 / (1.0 + np.exp(-(logits + np.array([fg_b[0], ug_b[0], sm_b[0]]))))
    fmean = float(sig[:, 0].mean())
    umean = float(sig[:, 1].mean())
    mix = sig[:, 2].astype(np.float32)                   # [B]

    xb = x.astype(bf16)
    x8 = np.clip(x, -240, 240).astype(fp8)
    mixs = (mix * OUT_SCALE).astype(np.float32)
    in_maps = []
    for i in range(NCORES):
        s = slice(i * BL, (i + 1) * BL)
        xTi = np.ascontiguousarray(xb[s].T)              # [D, BL]
        parts = []
        for j, g in enumerate(KGROUPS):
            n = len(g)
            xp = xTi[g[0] * P:(g[0] + n) * P].reshape(n, P, BL)
            parts.append(xp.transpose(1, 0, 2).reshape(P, n * BL))
            parts.append(we_parts[j])
        wx = np.ascontiguousarray(np.concatenate(parts, axis=1))
        in_maps.append({
            "wx": wx,
            "xn": np.ascontiguousarray(xb[s]),
            "x8": np.ascontiguousarray(x8[s].T).reshape(D // 2, 2 * BL),
            "m8": m8,
            "mx": np.ascontiguousarray(mixs[s].reshape(NB, P).T),
        })
    return in_maps, M, fmean, umean


def _finish(results, M, fmean, umean):
    out = np.concatenate([results[i]["out"] for i in range(NCORES)], axis=0)
    out *= 1.0 / OUT_SCALE
    delta = results[0]["dp"].astype(np.float64)
    for i in range(1, NCORES):
        delta += results[i]["dp"]
    delta_mean = (delta / B).astype(np.float32)
    new_M = fmean * M + umean * 0.1 * delta_mean
    return out.astype(np.float32), new_M.astype(np.float32)


def _run(inputs, trace=False, trace_kwargs=None):
    from concourse.bass_utils import run_bass_kernel_spmd

    nc = _build()
    in_maps, M, fmean, umean = _prepare(inputs)
    res = run_bass_kernel_spmd(nc, in_maps, core_ids=list(range(NCORES)),
                               trace=trace, **(trace_kwargs or {}))
    return _finish(res.results, M, fmean, umean), res


def kernel(**inputs):
    (out, new_M), _ = _run(inputs)
    return out, new_M


# revision 38
# speedup vs baseline: 1.0218x; 1.0218x over previous
"""Trainium2 Bass kernel for nn_ContinuumMemoryCell (scatter_memory).

Data-parallel over batch B across 8 NeuronCores. Device does the three
B-sized matmuls (error = x @ (V_w.T - M) in bf16, y_pred = x @ M in fp8
DoubleRow, and the Hebbian partial dp_i = error_i.T @ x_i in bf16) plus
the fused elementwise output out = y_pred + mix * error. Everything
O(D*H) or smaller (gate vectors, sigmoid means, the final new_M AXPY,
shard stitching) happens on host.

DMA strategy (measured): one HWDGE ring moves ~425 GB/s with big
transfers and parallel rings don't add bandwidth (HBM-bound), so all
input loads ride the sync ring as a few large chunks in need order;
out writes ride scalar's ring; dp writes ride gpsimd.
"""

import sys

if "/opt/trn_rl_repo" not in sys.path:
    sys.path.insert(0, "/opt/trn_rl_repo")

import numpy as np
import ml_dtypes

B, D, H = 16384, 1024, 1024
NCORES = 8
BL = B // NCORES          # 2048 batch rows per core
P = 128                   # partitions
NB = BL // P              # 16 b-tiles per core
NK = D // P               # 8 k-tiles (contraction over d)
NH = H // P               # 8 h-tiles (dp output rows)
FD = 512                  # matmul moving free-dim (one PSUM bank of f32)
OUT_SCALE = 4096.0        # 2^12: lifts M into fp8e4m3 normal range

# k-tile groups for the pass-A input stream: small leading chunks so the
# err pass starts early, larger trailing chunks to amortize per-transfer
# fixed cost.
KGROUPS = [[0], [1], [2, 3], [4, 5], [6, 7]]

_CACHE = {}


def _build():
    """Build + compile the SPMD Bass program (once per process)."""
    if "nc" in _CACHE:
        return _CACHE["nc"]

    import concourse.bacc as bacc
    import concourse.mybir as mybir
    import concourse.tile as tile

    bf16 = mybir.dt.bfloat16
    f32 = mybir.dt.float32
    fp8 = mybir.dt.float8e4

    nc = bacc.Bacc("TRN2", target_bir_lowering=False, debug=False,
                   num_devices=NCORES)

    # wx: per k-group [xT rows | we rows] pre-interleaved on host so the
    # whole pass-A stream is contiguous column chunks of one tensor.
    WXCOLS = NK * (BL + H)
    wx_d = nc.dram_tensor("wx", [P, WXCOLS], bf16, kind="ExternalInput")
    xn_d = nc.dram_tensor("xn", [BL, D], bf16, kind="ExternalInput")
    # x and M*2^12 in fp8 e4m3, pre-paired along d for DoubleRow matmuls:
    # row k' of the logical [D/2, 2, *] holds d = 2k' and 2k'+1.
    x8_d = nc.dram_tensor("x8", [D // 2, 2 * BL], fp8, kind="ExternalInput")
    m8_d = nc.dram_tensor("m8", [D // 2, 2 * D], fp8, kind="ExternalInput")
    mx_d = nc.dram_tensor("mx", [P, NB], f32, kind="ExternalInput")
    out_d = nc.dram_tensor("out", [BL, H], f32, kind="ExternalOutput")
    dp_d = nc.dram_tensor("dp", [H, D], f32, kind="ExternalOutput")

    with tile.TileContext(nc) as tc:
        with (
            tc.tile_pool(name="big", bufs=1) as big,
            tc.tile_pool(name="work", bufs=4) as work,
            tc.tile_pool(name="ps", bufs=4, space="PSUM") as ps,
        ):
            wx_g = [big.tile([P, len(g) * (BL + H)], bf16, tag=f"wxg{j}",
                             name=f"wxg{j}") for j, g in enumerate(KGROUPS)]
            xn_sb = big.tile([P, NB * D], bf16, tag="xn", name="xn")
            x8_sb = big.tile([P, 4 * 2 * BL], fp8, tag="x8", name="x8")
            m8_sb = big.tile([P, 4 * 2 * D], fp8, tag="m8", name="m8")
            err_sb = [big.tile([P, H], bf16, tag=f"err{i}", name=f"err{i}")
                      for i in range(NB)]
            mx_sb = big.tile([P, NB], f32, tag="mx", name="mx")

            # per-k views into the wx group tiles
            xT_v, we_v = {}, {}
            for j, g in enumerate(KGROUPS):
                n = len(g)
                for jj, k in enumerate(g):
                    xT_v[k] = (wx_g[j], jj * BL)
                    we_v[k] = (wx_g[j], n * BL + jj * H)

            # All input loads on the sync HWDGE ring, need order.
            off = 0
            for j, g in enumerate(KGROUPS):
                ncols = len(g) * (BL + H)
                nc.sync.dma_start(wx_g[j][:], wx_d[:, off:off + ncols])
                off += ncols
            nc.gpsimd.dma_start(mx_sb[:], mx_d[:])
            nc.sync.dma_start(
                x8_sb[:].rearrange("p (g c) -> p g c", g=4),
                x8_d.rearrange("(g p) c -> p g c", p=P))
            nc.sync.dma_start(
                m8_sb[:].rearrange("p (g c) -> p g c", g=4),
                m8_d.rearrange("(g p) c -> p g c", p=P))
            nc.sync.dma_start(
                xn_sb[:].rearrange("p (t d) -> p t d", t=NB),
                xn_d.rearrange("(t p) d -> p t d", p=P))

            # Phase 1a: err = x @ (V_w.T - M), stored bf16 in SBUF.
            for i in range(NB):
                pe = ps.tile([P, H], f32, tag="acc", name=f"pe{i}")
                for k in range(NK):
                    xt, xo = xT_v[k]
                    wt, wo = we_v[k]
                    lhs = xt[:, xo + i * P:xo + (i + 1) * P]
                    st, sp = (k == 0), (k == NK - 1)
                    for h2 in range(2):
                        nc.tensor.matmul(
                            pe[:, h2 * FD:(h2 + 1) * FD], lhs,
                            wt[:, wo + h2 * FD:wo + (h2 + 1) * FD],
                            start=st, stop=sp)
                nc.scalar.activation(err_sb[i][:], pe[:],
                                     mybir.ActivationFunctionType.Copy)

            # Phase 1b: y*2^12 = x @ (M*2^12) in fp8 DoubleRow (2 d-rows
            # per PE cell, half the instructions); epilogue out*2^12 =
            # err*(mix*2^12) + y*2^12, rescaled on host.
            x8_4 = x8_sb[:].rearrange("p (g two b) -> p g two b", g=4, two=2)
            m8_4 = m8_sb[:].rearrange("p (g two n) -> p g two n", g=4, two=2)
            for i in range(NB):
                py = ps.tile([P, D], f32, tag="acc", name=f"py{i}")
                for kg in range(4):
                    lhs3 = x8_4[:, kg, :, i * P:(i + 1) * P]
                    st, sp = (kg == 0), (kg == 3)
                    for h2 in range(2):
                        nc.tensor.matmul(
                            py[:, h2 * FD:(h2 + 1) * FD], lhs3,
                            m8_4[:, kg, :, h2 * FD:(h2 + 1) * FD],
                            start=st, stop=sp,
                            perf_mode=mybir.MatmulPerfMode.DoubleRow)
                o = work.tile([P, D], f32, tag="o", name=f"o{i}", bufs=6)
                for h2 in range(2):
                    nc.vector.scalar_tensor_tensor(
                        o[:, h2 * FD:(h2 + 1) * FD],
                        err_sb[i][:, h2 * FD:(h2 + 1) * FD],
                        mx_sb[:, i:i + 1],
                        py[:, h2 * FD:(h2 + 1) * FD],
                        mybir.AluOpType.mult, mybir.AluOpType.add)
                nc.scalar.dma_start(out_d[i * P:(i + 1) * P, :], o[:])

            # Phase 2: dp[h-tile] = sum_b err[b, h-tile].T @ x[b, :]
            for t in range(NH):
                pd = ps.tile([P, D], f32, tag="acc", name=f"pd{t}")
                for i in range(NB):
                    lhs = err_sb[i][:, t * P:(t + 1) * P]
                    st, sp = (i == 0), (i == NB - 1)
                    for h2 in range(2):
                        nc.tensor.matmul(
                            pd[:, h2 * FD:(h2 + 1) * FD], lhs,
                            xn_sb[:, i * D + h2 * FD:i * D + (h2 + 1) * FD],
                            start=st, stop=sp)
                dpt = work.tile([P, D], f32, tag="dpt", name=f"dpt{t}")
                dp_eng = [nc.sync, nc.scalar]
                if t < NH - 1:
                    for h2 in range(2):
                        nc.vector.tensor_copy(dpt[:, h2 * FD:(h2 + 1) * FD],
                                              pd[:, h2 * FD:(h2 + 1) * FD])
                        dp_eng[h2].dma_start(
                            dp_d[t * P:(t + 1) * P, h2 * FD:(h2 + 1) * FD],
                            dpt[:, h2 * FD:(h2 + 1) * FD])
                else:
                    # Last h-tile gates the kernel tail: drain in quarters
                    # across both idle HWDGE rings to shorten the chain.
                    Q = FD // 2
                    for q in range(4):
                        nc.vector.tensor_copy(dpt[:, q * Q:(q + 1) * Q],
                                              pd[:, q * Q:(q + 1) * Q])
                        dp_eng[q % 2].dma_start(
                            dp_d[t * P:(t + 1) * P, q * Q:(q + 1) * Q],
                            dpt[:, q * Q:(q + 1) * Q])

    nc.compile()
    _CACHE["nc"] = nc
    return nc


def _prepare(inputs):
    """Host-side preprocessing: shard + dtype-convert + gate math."""
    x = np.asarray(inputs["x"], np.float32)
    V_w = np.asarray(inputs["V_w"], np.float32)
    M = np.asarray(inputs["M"], np.float32)
    fg_w = np.asarray(inputs["fg_w"], np.float32)
    fg_b = np.asarray(inputs["fg_b"], np.float32)
    ug_w = np.asarray(inputs["ug_w"], np.float32)
    ug_b = np.asarray(inputs["ug_b"], np.float32)
    sm_w = np.asarray(inputs["sm_w"], np.float32)
    sm_b = np.asarray(inputs["sm_b"], np.float32)

    bf16 = ml_dtypes.bfloat16
    fp8 = ml_dtypes.float8_e4m3
    VT = V_w.T                                # [D, H]
    we = np.ascontiguousarray(VT - M).astype(bf16)
    m8 = np.clip(M * OUT_SCALE, -240, 240).astype(fp8).reshape(D // 2, 2 * D)

    # we parts per k-group, shared across cores: [P, n*H] each
    we_parts = []
    for g in KGROUPS:
        n = len(g)
        wp = we[g[0] * P:(g[0] + n) * P].reshape(n, P, H)
        we_parts.append(wp.transpose(1, 0, 2).reshape(P, n * H))

    # v @ a == x @ (V_w.T @ a): collapse each gate to one D-vector on x.
    c_f = VT @ fg_w[0, :H] + fg_w[0, H:]
    c_u = VT @ ug_w[0, :H] + ug_w[0, H:]
    c_m = VT @ sm_w[0]
    logits = x @ np.stack([c_f, c_u, c_m], axis=1)       # [B, 3]
    sig = 1.0 / (1.0 + np.exp(-(logits + np.array([fg_b[0], ug_b[0], sm_b[0]]))))
    fmean = float(sig[:, 0].mean())
    umean = float(sig[:, 1].mean())
    mix = sig[:, 2].astype(np.float32)                   # [B]

    xb = x.astype(bf16)
    x8 = np.clip(x, -240, 240).astype(fp8)
    mixs = (mix * OUT_SCALE).astype(np.float32)
    in_maps = []
    for i in range(NCORES):
        s = slice(i * BL, (i + 1) * BL)
        xTi = np.ascontiguousarray(xb[s].T)              # [D, BL]
        parts = []
        for j, g in enumerate(KGROUPS):
            n = len(g)
            xp = xTi[g[0] * P:(g[0] + n) * P].reshape(n, P, BL)
            parts.append(xp.transpose(1, 0, 2).reshape(P, n * BL))
            parts.append(we_parts[j])
        wx = np.ascontiguousarray(np.concatenate(parts, axis=1))
        in_maps.append({
            "wx": wx,
            "xn": np.ascontiguousarray(xb[s]),
            "x8": np.ascontiguousarray(x8[s].T).reshape(D // 2, 2 * BL),
            "m8": m8,
            "mx": np.ascontiguousarray(mixs[s].reshape(NB, P).T),
        })
    return in_maps, M, fmean, umean


def _finish(results, M, fmean, umean):
    out = np.concatenate([results[i]["out"] for i in range(NCORES)], axis=0)
    out *= 1.0 / OUT_SCALE
    delta = results[0]["dp"].astype(np.float64)
    for i in range(1, NCORES):
        delta += results[i]["dp"]
    delta_mean = (delta / B).astype(np.float32)
    new_M = fmean * M + umean * 0.1 * delta_mean
    return out.astype(np.float32), new_M.astype(np.float32)


def _run(inputs, trace=False, trace_kwargs=None):
    from concourse.bass_utils import run_bass_kernel_spmd

    nc = _build()
    in_maps, M, fmean, umean = _prepare(inputs)
    res = run_bass_kernel_spmd(nc, in_maps, core_ids=list(range(NCORES)),
                               trace=trace, **(trace_kwargs or {}))
    return _finish(res.results, M, fmean, umean), res


def kernel(**inputs):
    (out, new_M), _ = _run(inputs)
    return out, new_M
